# revision 14
# baseline (speedup 1.0000x reference)
"""Bass/Trainium2 kernel for nn_BlastocystAuxLoss.

Computes a masked MSE over B=16,777,216 elements:
    late stages are labels 8..15; target[s] = (s-8) * 4/7 for late stages;
    loss = sum_{s>=8} (x - target)^2 / count(s>=8)   (0.0 if count == 0)

Strategy: trivially data-parallel over 8 NeuronCores. Each core streams its
B/8 shard of blast_scores (f32) and stage_labels (i32) from HBM and computes
per-partition partial {count, sse} columns on-chip; the [128, 2*NT] partial
tile is DMA'd out and the final scalar reduction happens on host in f64.
No collectives needed.

Per-element identities (s = label, x = score):
    v  = relu(s - 8)                  (ACT; == 7/4 * target)
    m  = sigmoid(64*s - 480) in {0,1} (ACT; count accumulates for free)
    z  = 7/4*x - v                    (DVE scalar_tensor_tensor)
    zm = z * m                        (DVE tensor_tensor)
    sse += (4/7)^2 * zm^2             (DVE tensor_tensor_reduce, fused
                                       square + row-sum accumulate)

DMA layout: 4096-wide f32/i32 tiles (16 KiB contiguous per partition row)
measured at ~418 GB/s on this part vs ~340 GB/s for 8/64 KiB rows; the
tail tapers so the compute pipeline drains quickly after the last transfer.
"""

from contextlib import ExitStack

import numpy as np

B = 16777216
N_CORES = 8
SHARD = B // N_CORES  # 2,097,152
P = 128

_NC_CACHE = {}


def build_v2(shard=SHARD, sizes=None, ring=4, use_ttr=False, use_chunk=True):
    """Raw-Bass builder: hand-scheduled 3-engine pipeline (sync/ACT/DVE)."""
    import concourse.bacc as bacc
    from concourse import mybir

    free = shard // P
    if sizes is None:
        sizes = [4096, 4096, 4096, 2048, 1024, 512, 256, 256]
        if sum(sizes) != free:  # non-default shard (tests)
            fd8 = free // 8
            sizes = [fd8] * 8
    assert sum(sizes) == free
    fd = max(sizes)
    NT = len(sizes)
    offs = [sum(sizes[:i]) for i in range(NT)]
    R = min(ring, NT)

    nc = bacc.Bacc("TRN2", target_bir_lowering=False)
    x_ext = nc.declare_dram_parameter(
        "blast_scores", [shard], mybir.dt.float32, isOutput=False
    )
    s_ext = nc.declare_dram_parameter(
        "stage_labels", [shard], mybir.dt.int32, isOutput=False
    )
    out_ext = nc.declare_dram_parameter(
        "out", [P, 2 * NT], mybir.dt.float32, isOutput=True
    )

    # Per-tile contiguous-chunk views: tile k is the contiguous DRAM slice
    # [P*offs[k], P*(offs[k]+w)) seen as [P, w] (row stride w*4B). Fully
    # sequential HBM reads; the partition->element mapping differs per tile
    # but is irrelevant since everything reduces to global sums.
    def chunk(ext, k, w):
        if not use_chunk:
            v = ext.ap().rearrange("(p f) -> p f", p=P)
            return v[:, offs[k] : offs[k] + w]
        a = P * offs[k]
        return ext.ap()[a : a + P * w].rearrange("(p f) -> p f", p=P)

    c74 = 7.0 / 4.0
    csq = 16.0 / 49.0

    f32 = mybir.dt.float32
    i32 = mybir.dt.int32
    bf16 = mybir.dt.bfloat16
    Alu = mybir.AluOpType
    Act = mybir.ActivationFunctionType

    x_t = [nc.alloc_sbuf_tensor(f"x{i}", [P, fd], f32).ap() for i in range(R)]
    s_t = [nc.alloc_sbuf_tensor(f"s{i}", [P, fd], i32).ap() for i in range(R)]
    m_t = [nc.alloc_sbuf_tensor(f"m{i}", [P, fd], bf16).ap() for i in range(R)]
    v_t = [nc.alloc_sbuf_tensor(f"v{i}", [P, fd], bf16).ap() for i in range(2)]
    z_t = nc.alloc_sbuf_tensor("z", [P, fd], bf16).ap()
    zm_t = nc.alloc_sbuf_tensor("zm", [P, fd], bf16).ap()
    sq_t = nc.alloc_sbuf_tensor("sq", [P, fd], bf16).ap()
    # acc[:, k] = per-partition count of tile k; acc[:, NT+k] = partial sse
    acc = nc.alloc_sbuf_tensor("acc", [P, 2 * NT], f32).ap()
    # Per-partition bias constants for the two activations.
    relu_bias = nc.alloc_sbuf_tensor("relu_bias", [P, 1], f32).ap()
    sig_bias = nc.alloc_sbuf_tensor("sig_bias", [P, 1], f32).ap()

    with ExitStack() as ctx:
        dma_x = [ctx.enter_context(nc.semaphore(f"dma_x{i}")) for i in range(R)]
        dma_s = [ctx.enter_context(nc.semaphore(f"dma_s{i}")) for i in range(R)]
        dve = ctx.enter_context(nc.semaphore("dve"))
        act = ctx.enter_context(nc.semaphore("act"))
        outd = ctx.enter_context(nc.semaphore("outd"))
        bias_rdy = ctx.enter_context(nc.semaphore("bias_rdy"))
        block = ctx.enter_context(nc.Block())

        # Semaphore increment ledger:
        #   ACT: 2 per tile (v, m)                -> 2*NT total
        #   DVE: 3 per tile (z, zm, ttr)          -> 3*NT total
        #   DMA slot sems: +16 per transfer into that slot

        @block.sync
        def _(sync):
            for k in range(NT):
                i = k % R
                w = sizes[k]
                if k >= R:
                    # s slot free when v(k-R) and m(k-R) done (ACT consumed s)
                    sync.wait_ge(act, 2 * (k - R) + 2)
                    # x slot free when z(k-R) done
                    sync.wait_ge(dve, 3 * (k - R) + 1)
                sync.dma_start(
                    out=s_t[i][:, :w], in_=chunk(s_ext, k, w)
                ).then_inc(dma_s[i], 16)
                sync.dma_start(
                    out=x_t[i][:, :w], in_=chunk(x_ext, k, w)
                ).then_inc(dma_x[i], 16)
            # all partials written: ship acc (host does the final reduce)
            sync.wait_ge(act, 2 * NT)
            sync.wait_ge(dve, 3 * NT)
            sync.dma_start(out=out_ext.ap()[:, :], in_=acc[:, :]).then_inc(outd, 16)
            sync.wait_ge(outd, 16)

        @block.gpsimd
        def _(gpsimd):
            gpsimd.memset(relu_bias[:, :], -8.0).then_inc(bias_rdy, 1)
            gpsimd.memset(sig_bias[:, :], -480.0).then_inc(bias_rdy, 1)

        @block.scalar
        def _(scalar):
            scalar.wait_ge(bias_rdy, 2)
            for k in range(NT):
                i = k % R
                w = sizes[k]
                rnd = 16 * (k // R + 1)
                scalar.wait_ge(dma_s[i], rnd)
                # v = relu(s - 8)  (integer-exact in bf16; 7/4 * target)
                if k >= 2:
                    # v slot free when z(k-2) done
                    scalar.wait_ge(dve, 3 * (k - 2) + 1)
                scalar.activation(
                    v_t[k % 2][:, :w], s_t[i][:, :w], Act.Relu,
                    bias=relu_bias[:, :], scale=1.0,
                ).then_inc(act, 1)
                # m = sigmoid(64*s - 480) in {0,1}; count accumulates free
                if k >= R:
                    # m slot free when zm(k-R) done
                    scalar.wait_ge(dve, 3 * (k - R) + 2)
                scalar.activation(
                    m_t[i][:, :w], s_t[i][:, :w], Act.Sigmoid,
                    bias=sig_bias[:, :], scale=64.0,
                    accum_out=acc[:, k : k + 1],
                ).then_inc(act, 1)

        @block.vector
        def _(vector):
            for k in range(NT):
                i = k % R
                w = sizes[k]
                rnd = 16 * (k // R + 1)
                # z = 7/4*x - v  (single bf16 rounding per element)
                vector.wait_ge(dma_x[i], rnd)
                vector.wait_ge(act, 2 * k + 1)  # v(k) ready
                vector.scalar_tensor_tensor(
                    z_t[:, :w], x_t[i][:, :w], c74, v_t[k % 2][:, :w],
                    Alu.mult, Alu.subtract,
                ).then_inc(dve, 1)
                # zm = z * m (exact: m in {0,1})
                vector.wait_ge(act, 2 * k + 2)  # m(k) ready
                vector.tensor_tensor(
                    zm_t[:, :w], z_t[:, :w], m_t[i][:, :w], Alu.mult
                ).then_inc(dve, 1)
                # sq = (zm*zm) * 16/49; acc[:, NT+k] = row-sum(sq) fused
                if use_ttr:
                    vector.tensor_tensor_reduce(
                        sq_t[:, :w], zm_t[:, :w], zm_t[:, :w], csq, 0.0,
                        Alu.mult, Alu.add, accum_out=acc[:, NT + k : NT + k + 1],
                    ).then_inc(dve, 1)
                else:
                    vector.scalar_tensor_tensor(
                        sq_t[:, :w], zm_t[:, :w], csq, zm_t[:, :w],
                        Alu.mult, Alu.mult,
                        accum_out=acc[:, NT + k : NT + k + 1],
                    ).then_inc(dve, 1)

    nc.finalize()
    return nc


def run(x, s, **spmd_kwargs):
    """Shard, run on 8 cores, host-reduce. Returns (loss, BassKernelResults)."""
    from concourse.bass_utils import run_bass_kernel_spmd

    if "nc" not in _NC_CACHE:
        _NC_CACHE["nc"] = build_v2()
    nc = _NC_CACHE["nc"]

    in_maps = [
        {
            "blast_scores": x[i * SHARD : (i + 1) * SHARD],
            "stage_labels": s[i * SHARD : (i + 1) * SHARD],
        }
        for i in range(N_CORES)
    ]
    res = run_bass_kernel_spmd(nc, in_maps, core_ids=list(range(N_CORES)), **spmd_kwargs)

    cnt = 0.0
    sse = 0.0
    for r in res.results:
        o = r["out"].astype(np.float64).reshape(P, 2, -1)
        cnt += o[:, 0, :].sum()
        sse += o[:, 1, :].sum()
    val = sse / max(cnt, 1.0) if cnt > 0 else 0.0
    return np.asarray(val, dtype=np.float32), res


def kernel(**inputs):
    x = np.ascontiguousarray(np.asarray(inputs["blast_scores"], dtype=np.float32))
    s = np.ascontiguousarray(np.asarray(inputs["stage_labels"], dtype=np.int32))
    assert x.shape == (B,) and s.shape == (B,)
    return run(x, s)[0]
